# revision 11
# baseline (speedup 1.0000x reference)
"""Trainium2 Bass kernel for AudioVectorQuantizer (VQ codebook forward).

Data-parallel over 8 NeuronCores: each core handles 8192 of the 65536
tokens.  The [1024, 256] codebook is replicated.

Per core, per 128-token tile:
  - PE: fp32 matmul  dots2 = (2*codebook @ z_e_tile)^T  -> PSUM [128, 1024]
  - DVE: tensor_tensor_reduce fuses (dots2 - ||c||^2) eviction to SBUF with a
    running row-max (argmax of s = 2*x.c - ||c||^2  ==  argmin distance)
  - DVE: max_index extracts the argmax position (first occurrence, matching
    jnp.argmin tie-break)
  - GPSIMD dma_gather produces z_q = codebook[idx] (bit-exact rows)
  - ACT accumulates ||x||^2 partial sums; commitment loss is finished on the
    host: loss = (sum ||x||^2 - sum s_max) / (B*N*D)
"""

import numpy as np

import concourse.bass as bass  # noqa: F401  (keeps import side effects stable)
import concourse.mybir as mybir
import concourse.tile as tile
from concourse import bacc, bass_utils
from contextlib import ExitStack

B, N, D, K = 16, 4096, 256, 1024
NCORES = 8
TOK = B * N              # 65536
P = 128
NEG = -3.0e38

F32 = mybir.dt.float32
U16 = mybir.dt.uint16
I16 = mybir.dt.int16


def build_nc(shard):
    """Build the per-core Bass program for a shard of `shard` tokens."""
    ntile = shard // P                 # 128-token tiles
    chunk = min(2048, shard)           # tokens per z-load chunk
    nchunk = shard // chunk
    tiles_per_chunk = chunk // P
    gtok = min(1024, shard)            # tokens per gather group
    gtiles = gtok // P                 # tiles per group (8)
    ngroup = shard // gtok
    assert gtok % 128 == 0

    nc = bacc.Bacc("TRN2", target_bir_lowering=False, debug=False)

    z_et = nc.dram_tensor("z_et", [D, shard], F32, kind="ExternalInput")
    cb2t = nc.dram_tensor("cb2t", [D, K], F32, kind="ExternalInput")
    cnormb = nc.dram_tensor("cnormb", [P, K], F32, kind="ExternalInput")
    cbd = nc.dram_tensor("cb", [K, D], F32, kind="ExternalInput")

    zq = nc.dram_tensor("zq", [shard, D], F32, kind="ExternalOutput")
    idxd = nc.dram_tensor("idxd", [P, ntile], I16, kind="ExternalOutput")
    smaxd = nc.dram_tensor("smaxd", [P, ntile * 8], F32, kind="ExternalOutput")
    xnd = nc.dram_tensor("xnd", [P, 2 * nchunk], F32, kind="ExternalOutput")

    with tile.TileContext(nc) as tc, ExitStack() as ctx:
        const = ctx.enter_context(tc.tile_pool(name="const", bufs=1))
        zpool = ctx.enter_context(tc.tile_pool(name="z", bufs=2))
        xpool = ctx.enter_context(tc.tile_pool(name="xsc", bufs=2))
        psum = ctx.enter_context(tc.tile_pool(name="ps", bufs=3, space="PSUM"))
        dpool = ctx.enter_context(tc.tile_pool(name="dots", bufs=3))
        spool = ctx.enter_context(tc.tile_pool(name="s", bufs=3))
        gpool = ctx.enter_context(tc.tile_pool(name="zqg", bufs=2))
        dbpool = ctx.enter_context(tc.tile_pool(name="dbounce", bufs=1, space="DRAM"))

        cb2t_hi = const.tile([P, K], F32)
        nc.sync.dma_start(cb2t_hi[:], cb2t[0:P, :])
        cb2t_lo = const.tile([P, K], F32)
        nc.sync.dma_start(cb2t_lo[:], cb2t[P:D, :])
        cnb = const.tile([P, K], F32)
        nc.sync.dma_start(cnb[:], cnormb[:, :])

        idx_all = const.tile([P, ntile * 8], U16)
        smax8_all = const.tile([P, ntile * 8], F32)
        xn_all = const.tile([P, 2 * nchunk], F32)
        idx_c = const.tile([P, ntile], I16)
        idxg = const.tile([P, shard // 16], I16)     # gather-index layout
        nc.vector.memset(idxg[:], 0)
        idx_bounce = dbpool.tile([1, shard], I16)    # token-major flat

        def do_group(g):
            """Index shuffle + gather + store for tokens [g*gtok, (g+1)*gtok)."""
            # compact slot-0 of each tile's 8-wide max_index output
            src = idx_all[:, g * gtiles * 8:(g + 1) * gtiles * 8]
            src_v = src.rearrange("p (t e) -> p t e", e=8)[:, :, 0:1]
            nc.vector.tensor_copy(
                idx_c[:, g * gtiles:(g + 1) * gtiles], src_v
            )
            # bounce through DRAM (token-major) to reach the 16-partition
            # wrapped gather-index layout
            st_dst = idx_bounce[0:1, g * gtok:(g + 1) * gtok].rearrange(
                "o (t q) -> q (o t)", q=P
            )
            nc.sync.dma_start(st_dst, idx_c[:, g * gtiles:(g + 1) * gtiles])
            # wrapped reload: idxg[p, s] = idx of token s*16 + p.  The Q7
            # gather ucode farms positions out to 8 cores, each reading its
            # own 16-partition replica — so replicate to all 128 partitions.
            ld_src = idx_bounce[0:1, g * gtok:(g + 1) * gtok].rearrange(
                "o (s p) -> p (o s)", p=16
            )
            for rep in range(8):
                nc.sync.dma_start(
                    idxg[16 * rep:16 * (rep + 1),
                         g * (gtok // 16):(g + 1) * (gtok // 16)],
                    ld_src,
                )
            # gather codebook rows
            zq_g = gpool.tile([P, gtiles * D], F32, tag="zqg")
            zq_gv = zq_g[:].rearrange("p (t d) -> p t d", d=D)
            nc.gpsimd.dma_gather(
                zq_gv,
                cbd[:, :],
                idxg[:, g * (gtok // 16):(g + 1) * (gtok // 16)],
                gtok,
                gtok,
                D,
            )
            # store: token t = g*gtok + col*128 + p
            dst = zq[:, :].rearrange("(g t p) d -> g p t d", g=ngroup, p=P)[g]
            nc.sync.dma_start(dst, zq_gv)

        for c in range(nchunk):
            zhi = zpool.tile([P, chunk], F32, tag="zhi")
            nc.sync.dma_start(zhi[:], z_et[0:P, c * chunk:(c + 1) * chunk])
            zlo = zpool.tile([P, chunk], F32, tag="zlo")
            nc.sync.dma_start(zlo[:], z_et[P:D, c * chunk:(c + 1) * chunk])
            # ||x||^2 partial sums (per-partition over this chunk)
            xsc = xpool.tile([P, chunk], F32, tag="xsc")
            nc.scalar.activation(
                xsc[:], zhi[:], mybir.ActivationFunctionType.Square,
                accum_out=xn_all[:, 2 * c:2 * c + 1],
            )
            xsc2 = xpool.tile([P, chunk], F32, tag="xsc")
            nc.scalar.activation(
                xsc2[:], zlo[:], mybir.ActivationFunctionType.Square,
                accum_out=xn_all[:, 2 * c + 1:2 * c + 2],
            )
            for tj in range(tiles_per_chunk):
                j = c * tiles_per_chunk + tj
                lhs_hi = zhi[:, tj * P:(tj + 1) * P]
                lhs_lo = zlo[:, tj * P:(tj + 1) * P]
                s2 = psum.tile([P, K], F32, tag="s2")
                nc.tensor.matmul(s2[:, 0:512], lhs_hi, cb2t_hi[:, 0:512],
                                 start=True, stop=False)
                nc.tensor.matmul(s2[:, 0:512], lhs_lo, cb2t_lo[:, 0:512],
                                 start=False, stop=True)
                nc.tensor.matmul(s2[:, 512:1024], lhs_hi, cb2t_hi[:, 512:1024],
                                 start=True, stop=False)
                nc.tensor.matmul(s2[:, 512:1024], lhs_lo, cb2t_lo[:, 512:1024],
                                 start=False, stop=True)

                dots_sb = dpool.tile([P, K], F32, tag="dots_sb")
                nc.scalar.copy(dots_sb[:], s2[:])
                s_sb = spool.tile([P, K], F32, tag="s_sb")
                nc.gpsimd.tensor_sub(s_sb[:], dots_sb[:], cnb[:])
                m8 = smax8_all[:, j * 8:(j + 1) * 8]
                nc.vector.max(m8, s_sb[:])
                nc.vector.max_index(idx_all[:, j * 8:(j + 1) * 8], m8, s_sb[:])

            # groups fully contained in this chunk are ready
            for g in range(c * chunk // gtok, (c + 1) * chunk // gtok):
                do_group(g)

        # small outputs
        nc.sync.dma_start(idxd[:, :], idx_c[:, :])
        nc.sync.dma_start(smaxd[:, :], smax8_all[:, :])
        nc.sync.dma_start(xnd[:, :], xn_all[:, :])

    nc.compile()
    return nc


def make_in_maps(z_e, codebook, ncores=NCORES):
    z_e = np.asarray(z_e, dtype=np.float32)
    cb = np.ascontiguousarray(np.asarray(codebook, dtype=np.float32))
    tok = z_e.shape[0] * z_e.shape[1]
    shard = tok // ncores
    flat = z_e.reshape(tok, D)
    cb2t = np.ascontiguousarray((2.0 * cb).T)
    cnorm = (cb.astype(np.float64) ** 2).sum(1).astype(np.float32)
    cnormb = np.ascontiguousarray(np.broadcast_to(cnorm[None, :], (P, K)))
    in_maps = []
    for s in range(ncores):
        zsh = np.ascontiguousarray(flat[s * shard:(s + 1) * shard].T)
        in_maps.append({"z_et": zsh, "cb2t": cb2t, "cnormb": cnormb, "cb": cb})
    return in_maps, shard


def assemble_outputs(outs, z_e_shape, shard, ncores=NCORES):
    """outs: list of per-core dicts with zq/idxd/smaxd/xnd."""
    tok = z_e_shape[0] * z_e_shape[1]
    zq = np.concatenate([outs[s]["zq"] for s in range(ncores)], axis=0)
    zq = zq.reshape(z_e_shape)
    idx_full = np.empty(tok, np.int32)
    smax_sum = 0.0
    xn_sum = 0.0
    for s in range(ncores):
        idx16 = outs[s]["idxd"]                      # [128, ntile] int16
        idx_full[s * shard:(s + 1) * shard] = (
            idx16.T.reshape(shard).astype(np.int32)
        )
        smax_sum += outs[s]["smaxd"][:, 0::8].astype(np.float64).sum()
        xn_sum += outs[s]["xnd"].astype(np.float64).sum()
    indices = idx_full.reshape(z_e_shape[:2])
    loss = np.float32((xn_sum - smax_sum) / (tok * D))
    return zq, indices, loss


_NC_CACHE = {}


def _get_nc(shard):
    if shard not in _NC_CACHE:
        _NC_CACHE[shard] = build_nc(shard)
    return _NC_CACHE[shard]


def kernel(z_e, codebook):
    z_e = np.asarray(z_e, dtype=np.float32)
    in_maps, shard = make_in_maps(z_e, codebook)
    nc = _get_nc(shard)
    res = bass_utils.run_bass_kernel_spmd(nc, in_maps, core_ids=list(range(NCORES)))
    return assemble_outputs(res.results, z_e.shape, shard)


# revision 12
# speedup vs baseline: 62.7992x; 62.7992x over previous
"""Trainium2 Bass kernel for AudioVectorQuantizer (VQ codebook forward).

Data-parallel over 8 NeuronCores: each core handles 8192 of the 65536
tokens.  The [1024, 256] codebook is replicated.

Per core, per 128-token tile:
  - PE: fp32 matmul  dots2 = (2*codebook @ z_e_tile)^T  -> PSUM [128, 1024]
  - DVE: tensor_tensor_reduce fuses (dots2 - ||c||^2) eviction to SBUF with a
    running row-max (argmax of s = 2*x.c - ||c||^2  ==  argmin distance)
  - DVE: max_index extracts the argmax position (first occurrence, matching
    jnp.argmin tie-break)
  - GPSIMD dma_gather produces z_q = codebook[idx] (bit-exact rows)
  - ACT accumulates ||x||^2 partial sums; commitment loss is finished on the
    host: loss = (sum ||x||^2 - sum s_max) / (B*N*D)
"""

import numpy as np

import concourse.bass as bass  # noqa: F401  (keeps import side effects stable)
import concourse.mybir as mybir
import concourse.tile as tile
from concourse import bacc, bass_utils
from contextlib import ExitStack

B, N, D, K = 16, 4096, 256, 1024
NCORES = 8
TOK = B * N              # 65536
P = 128
NEG = -3.0e38

F32 = mybir.dt.float32
U16 = mybir.dt.uint16
I16 = mybir.dt.int16


def build_nc(shard):
    """Build the per-core Bass program for a shard of `shard` tokens."""
    ntile = shard // P                 # 128-token tiles
    chunk = min(2048, shard)           # tokens per z-load chunk
    nchunk = shard // chunk
    tiles_per_chunk = chunk // P
    gtok = min(1024, shard)            # tokens per gather group
    gtiles = gtok // P                 # tiles per group (8)
    ngroup = shard // gtok
    assert gtok % 128 == 0

    nc = bacc.Bacc("TRN2", target_bir_lowering=False, debug=False)

    z_et = nc.dram_tensor("z_et", [D, shard], F32, kind="ExternalInput")
    cb2t = nc.dram_tensor("cb2t", [D, K], F32, kind="ExternalInput")
    cnormb = nc.dram_tensor("cnormb", [P, K], F32, kind="ExternalInput")
    cbd = nc.dram_tensor("cb", [K, D], F32, kind="ExternalInput")

    zq = nc.dram_tensor("zq", [shard, D], F32, kind="ExternalOutput")
    idxd = nc.dram_tensor("idxd", [P, ntile], I16, kind="ExternalOutput")
    smaxd = nc.dram_tensor("smaxd", [P, ntile * 8], F32, kind="ExternalOutput")
    xnd = nc.dram_tensor("xnd", [P, 2 * nchunk], F32, kind="ExternalOutput")

    with tile.TileContext(nc) as tc, ExitStack() as ctx:
        const = ctx.enter_context(tc.tile_pool(name="const", bufs=1))
        zpool = ctx.enter_context(tc.tile_pool(name="z", bufs=2))
        xpool = ctx.enter_context(tc.tile_pool(name="xsc", bufs=2))
        psum = ctx.enter_context(tc.tile_pool(name="ps", bufs=3, space="PSUM"))
        dpool = ctx.enter_context(tc.tile_pool(name="dots", bufs=3))
        spool = ctx.enter_context(tc.tile_pool(name="s", bufs=3))
        gpool = ctx.enter_context(tc.tile_pool(name="zqg", bufs=2))
        dbpool = ctx.enter_context(tc.tile_pool(name="dbounce", bufs=1, space="DRAM"))

        cb2t_hi = const.tile([P, K], F32)
        nc.sync.dma_start(cb2t_hi[:], cb2t[0:P, :])
        cb2t_lo = const.tile([P, K], F32)
        nc.sync.dma_start(cb2t_lo[:], cb2t[P:D, :])
        cnb = const.tile([P, K], F32)
        nc.sync.dma_start(cnb[:], cnormb[:, :])

        idx_all = const.tile([P, ntile * 8], U16)
        smax8_all = const.tile([P, ntile * 8], F32)
        xn_all = const.tile([P, 2 * nchunk], F32)
        idx_c = const.tile([P, ntile], I16)
        idxg = const.tile([P, shard // 16], I16)     # gather-index layout
        nc.vector.memset(idxg[:], 0)
        idx_bounce = dbpool.tile([1, shard], I16)    # token-major flat

        def do_group(g):
            """Index shuffle + gather + store for tokens [g*gtok, (g+1)*gtok)."""
            # compact slot-0 of each tile's 8-wide max_index output
            src = idx_all[:, g * gtiles * 8:(g + 1) * gtiles * 8]
            src_v = src.rearrange("p (t e) -> p t e", e=8)[:, :, 0:1]
            nc.vector.tensor_copy(
                idx_c[:, g * gtiles:(g + 1) * gtiles], src_v
            )
            # bounce through DRAM (token-major) to reach the 16-partition
            # wrapped gather-index layout
            st_dst = idx_bounce[0:1, g * gtok:(g + 1) * gtok].rearrange(
                "o (t q) -> q (o t)", q=P
            )
            nc.sync.dma_start(st_dst, idx_c[:, g * gtiles:(g + 1) * gtiles])
            # wrapped reload: idxg[p, s] = idx of token s*16 + p.  The Q7
            # gather ucode farms positions out to 8 cores, each reading its
            # own 16-partition replica — so replicate to all 128 partitions.
            ld_src = idx_bounce[0:1, g * gtok:(g + 1) * gtok].rearrange(
                "o (s p) -> p (o s)", p=16
            )
            for rep in range(8):
                nc.sync.dma_start(
                    idxg[16 * rep:16 * (rep + 1),
                         g * (gtok // 16):(g + 1) * (gtok // 16)],
                    ld_src,
                )
            # gather codebook rows
            zq_g = gpool.tile([P, gtiles * D], F32, tag="zqg")
            zq_gv = zq_g[:].rearrange("p (t d) -> p t d", d=D)
            nc.gpsimd.dma_gather(
                zq_gv,
                cbd[:, :],
                idxg[:, g * (gtok // 16):(g + 1) * (gtok // 16)],
                gtok,
                gtok,
                D,
            )
            # store: token t = g*gtok + col*128 + p
            dst = zq[:, :].rearrange("(g t p) d -> g p t d", g=ngroup, p=P)[g]
            nc.sync.dma_start(dst, zq_gv)

        ldq = 512                      # tokens per z-load sub-DMA
        for c in range(nchunk):
            zhi = zpool.tile([P, chunk], F32, tag="zhi")
            zlo = zpool.tile([P, chunk], F32, tag="zlo")
            for q in range(chunk // ldq):
                lo, hi = q * ldq, (q + 1) * ldq
                nc.sync.dma_start(zhi[:, lo:hi],
                                  z_et[0:P, c * chunk + lo:c * chunk + hi])
                nc.sync.dma_start(zlo[:, lo:hi],
                                  z_et[P:D, c * chunk + lo:c * chunk + hi])
            # ||x||^2 partial sums (per-partition over this chunk)
            xsc = xpool.tile([P, chunk], F32, tag="xsc")
            nc.scalar.activation(
                xsc[:], zhi[:], mybir.ActivationFunctionType.Square,
                accum_out=xn_all[:, 2 * c:2 * c + 1],
            )
            xsc2 = xpool.tile([P, chunk], F32, tag="xsc")
            nc.scalar.activation(
                xsc2[:], zlo[:], mybir.ActivationFunctionType.Square,
                accum_out=xn_all[:, 2 * c + 1:2 * c + 2],
            )
            for tj in range(tiles_per_chunk):
                j = c * tiles_per_chunk + tj
                lhs_hi = zhi[:, tj * P:(tj + 1) * P]
                lhs_lo = zlo[:, tj * P:(tj + 1) * P]
                s2 = psum.tile([P, K], F32, tag="s2")
                nc.tensor.matmul(s2[:, 0:512], lhs_hi, cb2t_hi[:, 0:512],
                                 start=True, stop=False)
                nc.tensor.matmul(s2[:, 0:512], lhs_lo, cb2t_lo[:, 0:512],
                                 start=False, stop=True)
                nc.tensor.matmul(s2[:, 512:1024], lhs_hi, cb2t_hi[:, 512:1024],
                                 start=True, stop=False)
                nc.tensor.matmul(s2[:, 512:1024], lhs_lo, cb2t_lo[:, 512:1024],
                                 start=False, stop=True)

                dots_sb = dpool.tile([P, K], F32, tag="dots_sb")
                nc.scalar.copy(dots_sb[:], s2[:])
                s_sb = spool.tile([P, K], F32, tag="s_sb")
                nc.gpsimd.tensor_sub(s_sb[:], dots_sb[:], cnb[:])
                m8 = smax8_all[:, j * 8:(j + 1) * 8]
                nc.vector.max(m8, s_sb[:])
                nc.vector.max_index(idx_all[:, j * 8:(j + 1) * 8], m8, s_sb[:])

            # groups fully contained in this chunk are ready
            for g in range(c * chunk // gtok, (c + 1) * chunk // gtok):
                do_group(g)

        # small outputs
        nc.sync.dma_start(idxd[:, :], idx_c[:, :])
        nc.sync.dma_start(smaxd[:, :], smax8_all[:, :])
        nc.sync.dma_start(xnd[:, :], xn_all[:, :])

    nc.compile()
    return nc


def make_in_maps(z_e, codebook, ncores=NCORES):
    z_e = np.asarray(z_e, dtype=np.float32)
    cb = np.ascontiguousarray(np.asarray(codebook, dtype=np.float32))
    tok = z_e.shape[0] * z_e.shape[1]
    shard = tok // ncores
    flat = z_e.reshape(tok, D)
    cb2t = np.ascontiguousarray((2.0 * cb).T)
    cnorm = (cb.astype(np.float64) ** 2).sum(1).astype(np.float32)
    cnormb = np.ascontiguousarray(np.broadcast_to(cnorm[None, :], (P, K)))
    in_maps = []
    for s in range(ncores):
        zsh = np.ascontiguousarray(flat[s * shard:(s + 1) * shard].T)
        in_maps.append({"z_et": zsh, "cb2t": cb2t, "cnormb": cnormb, "cb": cb})
    return in_maps, shard


def assemble_outputs(outs, z_e_shape, shard, ncores=NCORES):
    """outs: list of per-core dicts with zq/idxd/smaxd/xnd."""
    tok = z_e_shape[0] * z_e_shape[1]
    zq = np.concatenate([outs[s]["zq"] for s in range(ncores)], axis=0)
    zq = zq.reshape(z_e_shape)
    idx_full = np.empty(tok, np.int32)
    smax_sum = 0.0
    xn_sum = 0.0
    for s in range(ncores):
        idx16 = outs[s]["idxd"]                      # [128, ntile] int16
        idx_full[s * shard:(s + 1) * shard] = (
            idx16.T.reshape(shard).astype(np.int32)
        )
        smax_sum += outs[s]["smaxd"][:, 0::8].astype(np.float64).sum()
        xn_sum += outs[s]["xnd"].astype(np.float64).sum()
    indices = idx_full.reshape(z_e_shape[:2])
    loss = np.float32((xn_sum - smax_sum) / (tok * D))
    return zq, indices, loss


_NC_CACHE = {}


def _get_nc(shard):
    if shard not in _NC_CACHE:
        _NC_CACHE[shard] = build_nc(shard)
    return _NC_CACHE[shard]


def kernel(z_e, codebook):
    z_e = np.asarray(z_e, dtype=np.float32)
    in_maps, shard = make_in_maps(z_e, codebook)
    nc = _get_nc(shard)
    res = bass_utils.run_bass_kernel_spmd(nc, in_maps, core_ids=list(range(NCORES)))
    return assemble_outputs(res.results, z_e.shape, shard)
